# revision 35
# baseline (speedup 1.0000x reference)
"""FECAM layer Trainium2 kernel (bf16 matmul version).

Reference computation (per batch element b, X = x[b] in R^{512x512}, layout [l, c]):
    xp   = X^T                                  # [c, l]
    freq = xp @ D^T                             # DCT-II along l      [c, k]
    sd   = LN(freq) * gamma + beta              # LayerNorm over k
    h    = relu(sd @ W1^T)                      # [c, 2C]
    fw   = sigmoid(h @ W2^T)                    # [c, k]
    fw   = LN(fw) * gamma + beta
    out  = (xp * fw)^T = X .* fw^T              # [l, c]  (natural layout)

Device strategy (data parallel, 16 batch elements per core x 8 cores):
  - ALL matmul/transpose operands bf16 (measured end-to-end rel err ~5e-3
    vs 2e-2 gate).  MATMUL stays 1 cyc/row (same as f32r) but LDWEIGHTS
    drops 4x (f32 weights load at 4 cyc/row) so the PE queue never stalls
    on weight loads, and PE transposes drop 1.5 -> 1.0 cyc/row.
  - freq[c,k] = matmul(lhsT=x_b tiles [l,c], rhs=D^T tiles [l,k]) -> psum
  - LN1 stats via bn_stats/bn_aggr per group, aggregated into mvall
    [P, 2, KT]; ONE batched Ln + ONE batched Exp on [P, KT] per LN per
    batch (instead of 2 small ACT ops per group) -> 12 fewer ACT
    instructions per batch.  rstd = Exp(-0.5*Ln(var+eps)).
  - LN1 evict z = (pf - mu)*rstd on DVE tensor_scalar (psum fp32 1x),
    output bf16.
  - LN1 gamma/beta folded into fc1 on host: W1g[h,k]=w1[h,k]*gamma[k],
    b1[h]=sum_k beta[k]*w1[h,k]
  - z transposed 128x128 via PE (bf16, 1 cyc/row) into bf16 psum; evicted
    by ACT copy to zT [k,c]
  - fc1: hT = relu(W1g @ zT + b1) in [h,c] (ACT evict w/ per-part bias)
  - fc2: y = hT^T @ W2^T -> [c,k]; sigmoid = recip_approx_fast(1+Exp(-y))
  - LN2 stats likewise batched; z2 evict on DVE tensor_scalar (SBUF fp32
    2x mode), output bf16; transpose via PE; final affine (gamma/beta
    per-partition) on ACT; multiply by x on DVE
  - emission is software-pipelined with a 2-batch skew so the PE queue
    always has independent matmul work:
      cycle b emits: DCT+LN1(b) | T1(b-1) | fc1(b-1) x T2+final(b-2) | fc2(b-1)
"""

import sys

if "/opt/trn_rl_repo" not in sys.path:
    sys.path.insert(0, "/opt/trn_rl_repo")

import numpy as np

P = 128
C = 512          # channels == seq len == dct size
H = 1024         # hidden
CT = C // P      # 4 c-tiles
KT = C // P      # 4 k-tiles
HT = H // P      # 8 h-tiles
EPS = 1e-6
N_CORES = 8
B_FULL = 128

_NC_CACHE: dict = {}


def _build(nb: int):
    import concourse.bass as bass
    from concourse import bacc
    import concourse.mybir as mybir
    from concourse.tile import TileContext

    f32 = mybir.dt.float32
    bf16 = mybir.dt.bfloat16
    Relu = mybir.ActivationFunctionType.Relu
    Ln = mybir.ActivationFunctionType.Ln
    Exp = mybir.ActivationFunctionType.Exp
    Ident = mybir.ActivationFunctionType.Identity
    mult = mybir.AluOpType.mult
    sub = mybir.AluOpType.subtract

    mdt = bf16

    nc = bacc.Bacc()
    x_d = nc.declare_dram_parameter("x", [nb, C, C], mdt, isOutput=False)
    # xf rows: [e2 (128) | o2 (128) | o (256)] — host-folded DCT butterflies
    xf_d = nc.declare_dram_parameter("xf", [nb, C, C], mdt, isOutput=False)
    # dm row-block 0: [De2T | Do2T | DoT rows 0:128]; block 1: [pad | DoT 128:256]
    dm_d = nc.declare_dram_parameter("dm", [2 * P, C], mdt, isOutput=False)
    w1t_d = nc.declare_dram_parameter("w1t", [C, H], mdt, isOutput=False)
    b1_d = nc.declare_dram_parameter("b1", [H], f32, isOutput=False)
    w2t_d = nc.declare_dram_parameter("w2t", [H, C], mdt, isOutput=False)
    gb_d = nc.declare_dram_parameter("gb", [C, 2], f32, isOutput=False)
    id_d = nc.declare_dram_parameter("iden", [P, P], mdt, isOutput=False)
    out_d = nc.declare_dram_parameter("out", [nb, C, C], f32, isOutput=True)

    with TileContext(nc) as tc, \
            tc.tile_pool(name="consts", bufs=1) as consts, \
            tc.tile_pool(name="xin", bufs=4) as xin, \
            tc.tile_pool(name="work", bufs=2) as work, \
            tc.tile_pool(name="small", bufs=8) as small, \
            tc.tile_pool(name="res", bufs=4) as resp, \
            tc.tile_pool(name="ps_mm", bufs=4, space="PSUM") as ps_mm, \
            tc.tile_pool(name="ps_t1", bufs=1, space="PSUM") as ps_t1, \
            tc.tile_pool(name="ps_t2", bufs=1, space="PSUM") as ps_t2, \
            tc.tile_pool(name="ps_hw", bufs=2, space="PSUM") as ps_hw:

        # one ACT table set covering Ln/Exp/Identity/Copy/Relu: pre-seed it so
        # bacc's availability pass never inserts another load
        from concourse.hw_specs import get_activation_tables
        set_names = list(get_activation_tables(nc.m.arch))
        nc.scalar.add_instruction(mybir.InstLoadActFuncSet(
            name=nc.get_next_instruction_name(),
            act_func_set_id=set_names.index("natural_log_exp_and_others"),
            ins=[], outs=[]))

        dm_sb = consts.tile([P, 2, C], mdt)
        w1t_sb = consts.tile([P, KT, H], mdt)
        w2t_sb = consts.tile([P, HT, C], mdt)
        b1_sb = consts.tile([P, HT], f32)
        nc.sync.dma_start(out=b1_sb, in_=b1_d.rearrange("(t p) -> p t", p=P))
        gb_sb = consts.tile([P, KT, 2], f32)
        nc.sync.dma_start(out=gb_sb, in_=gb_d.rearrange("(t p) g -> p t g", p=P))
        id_sb = consts.tile([P, P], mdt)
        nc.sync.dma_start(out=id_sb, in_=id_d[:])
        eps_sb = consts.tile([P, 1], f32)
        nc.vector.memset(eps_sb, EPS)
        # persistent 2-slot psum scratch for each transpose stream: bf16
        # tiles are half a bank, so one [P,2,C] tile packs both slots into
        # a single bank (separate pools per bank would cost 4 banks)
        pt1_buf = ps_t1.tile([P, 2, C], mdt)
        pt2_buf = ps_t2.tile([P, 2, C], mdt)

        st: dict = {}   # per-batch live tiles

        def emit_load(b):
            xb = xin.tile([P, KT, C], mdt, tag="xb")
            xfb = xin.tile([P, KT, C], mdt, tag="xfb")
            if b == 0:
                # interleave dm/xf chunk loads across DMA queues so the first
                # DCT matmul (needs xfb[:,0] + dm cols 0:128) starts asap;
                # x(0) is only needed by the final multiply two cycles later
                nc.sync.dma_start(out=xfb[:, 0, :], in_=xf_d[b, 0:P, :])
                nc.sync.dma_start(out=dm_sb[:, 0, 0:P], in_=dm_d[0:P, 0:P])
                nc.sync.dma_start(out=dm_sb[:, 0, P:2 * P],
                                  in_=dm_d[0:P, P:2 * P])
                nc.sync.dma_start(out=dm_sb[:, 0, 2 * P:],
                                  in_=dm_d[0:P, 2 * P:])
                nc.sync.dma_start(out=dm_sb[:, 1, :], in_=dm_d[P:2 * P, :])
                for lt in range(1, KT):
                    nc.sync.dma_start(out=xfb[:, lt, :],
                                      in_=xf_d[b, lt * P:(lt + 1) * P, :])
                nc.sync.dma_start(out=xb,
                                  in_=x_d[b].rearrange("(t p) c -> p t c", p=P))
            else:
                nc.sync.dma_start(out=xfb,
                                  in_=xf_d[b].rearrange("(t p) c -> p t c", p=P))
                nc.sync.dma_start(out=xb,
                                  in_=x_d[b].rearrange("(t p) c -> p t c", p=P))
            st[b] = {"xb": xb, "xfb": xfb}

        def emit_dct_group(b, mc):
            """DCT matmul group mc -> psum pf; bn_stats/aggr into mvall."""
            if mc == 0:
                st[b]["pf"] = []
                mvall = small.tile([P, 2, KT], f32, tag="mvall")
                st[b]["mvall"] = mvall
            xfb = st[b]["xfb"]
            pf = ps_mm.tile([P, C], f32, tag="pf")
            st[b]["pf"].append(pf)
            cs = slice(mc * P, (mc + 1) * P)
            # folded DCT: freq[4k''] from e2, freq[4k''+2] from o2,
            # freq[2k'+1] from o (k-permutation absorbed into w1t rows)
            nc.tensor.matmul(pf[:, 0:P], lhsT=xfb[:, 0, cs],
                             rhs=dm_sb[:, 0, 0:P], start=True, stop=True)
            nc.tensor.matmul(pf[:, P:2 * P], lhsT=xfb[:, 1, cs],
                             rhs=dm_sb[:, 0, P:2 * P], start=True, stop=True)
            nc.tensor.matmul(pf[:, 2 * P:], lhsT=xfb[:, 2, cs],
                             rhs=dm_sb[:, 0, 2 * P:], start=True, stop=False)
            nc.tensor.matmul(pf[:, 2 * P:], lhsT=xfb[:, 3, cs],
                             rhs=dm_sb[:, 1, 2 * P:], start=False, stop=True)
            stats = small.tile([P, 6], f32, tag="stats")
            nc.vector.bn_stats(out=stats, in_=pf)
            nc.vector.bn_aggr(out=st[b]["mvall"][:, :, mc], in_=stats)

        def emit_ln1_rstd(b):
            """Batched rstd for all 4 groups: [P,KT] Ln + Exp."""
            mvall = st[b]["mvall"]
            lv = small.tile([P, KT], f32, tag="lv")
            nc.scalar.activation(out=lv, in_=mvall[:, 1, :], func=Ln,
                                 bias=eps_sb, scale=1.0)
            rstd = small.tile([P, KT], f32, tag="rstd")
            nc.scalar.activation(out=rstd, in_=lv, func=Exp,
                                 bias=0.0, scale=-0.5)
            st[b]["rstd"] = rstd

        def emit_ln1_evict(b, mc):
            if mc == 0:
                z_new = work.tile([P, CT, C], mdt, tag="z")
                st[b]["z"] = z_new
            mvall = st[b]["mvall"]
            rstd = st[b]["rstd"]
            nc.vector.tensor_scalar(out=st[b]["z"][:, mc, :],
                                    in0=st[b]["pf"][mc],
                                    scalar1=mvall[:, 0:1, mc], scalar2=rstd[:, mc:mc + 1],
                                    op0=sub, op1=mult)
            if mc == CT - 1:
                del st[b]["pf"], st[b]["mvall"], st[b]["rstd"]

        def emit_t1_group(b, kt):
            if "zT" not in st[b]:
                zT_new = work.tile([P, KT, C], mdt, tag="zT")
                st[b]["zT"] = zT_new
                st[b]["t1done"] = 0
            z = st[b]["z"]
            zT = st[b]["zT"]
            pt = pt1_buf[:, kt % 2, :]
            for mc in range(CT):
                nc.tensor.transpose(pt[:, mc * P:(mc + 1) * P],
                                    z[:, mc, kt * P:(kt + 1) * P], id_sb)
            nc.scalar.copy(out=zT[:, kt, :], in_=pt)
            st[b]["t1done"] += 1
            if st[b]["t1done"] == KT:
                del st[b]["z"]
                del st[b]["t1done"]

        def emit_fc1_group(b, mh):
            if mh == 0:
                hT_new = work.tile([P, HT, C], mdt, tag="hT")
                st[b]["hT"] = hT_new
            zT = st[b]["zT"]
            hT = st[b]["hT"]
            ph = ps_hw.tile([P, C], f32, tag="phw")
            for kt in range(KT):
                nc.tensor.matmul(
                    ph,
                    lhsT=w1t_sb[:, kt, mh * P:(mh + 1) * P],
                    rhs=zT[:, kt, :],
                    start=(kt == 0),
                    stop=(kt == KT - 1),
                )
            nc.scalar.activation(out=hT[:, mh, :], in_=ph, func=Relu,
                                 bias=b1_sb[:, mh:mh + 1], scale=1.0)
            if mh == HT - 1:
                del st[b]["zT"]

        def emit_fc2_group(b, mc):
            """fc2 matmuls + sigmoid + bn stats for group mc."""
            if mc == 0:
                fwp_new = work.tile([P, CT, C], f32, tag="fwp")
                st[b]["fwp"] = fwp_new
                mvall2 = small.tile([P, 2, KT], f32, tag="mvall")
                st[b]["mvall2"] = mvall2
            hT = st[b]["hT"]
            pw = ps_hw.tile([P, C], f32, tag="phw")
            for ht in range(HT):
                nc.tensor.matmul(
                    pw,
                    lhsT=hT[:, ht, mc * P:(mc + 1) * P],
                    rhs=w2t_sb[:, ht, :],
                    start=(ht == 0),
                    stop=(ht == HT - 1),
                )
            fwp = st[b]["fwp"]
            nc.scalar.activation(out=fwp[:, mc, :], in_=pw, func=Exp,
                                 bias=0.0, scale=-1.0)
            nc.vector.tensor_scalar_add(out=fwp[:, mc, :], in0=fwp[:, mc, :],
                                        scalar1=1.0)
            nc.vector.reciprocal_approx_fast(out=fwp[:, mc, :], in_=fwp[:, mc, :])
            stats2 = small.tile([P, 6], f32, tag="stats")
            nc.vector.bn_stats(out=stats2, in_=fwp[:, mc, :])
            nc.vector.bn_aggr(out=st[b]["mvall2"][:, :, mc], in_=stats2)
            if mc == CT - 1:
                del st[b]["hT"]

        def emit_ln2_rstd(b):
            mvall2 = st[b]["mvall2"]
            lv = small.tile([P, KT], f32, tag="lv")
            nc.scalar.activation(out=lv, in_=mvall2[:, 1, :], func=Ln,
                                 bias=eps_sb, scale=1.0)
            rstd2 = small.tile([P, KT], f32, tag="rstd")
            nc.scalar.activation(out=rstd2, in_=lv, func=Exp,
                                 bias=0.0, scale=-0.5)
            st[b]["rstd2"] = rstd2

        def emit_ln2_evict(b, mc):
            if mc == 0:
                z2_new = work.tile([P, CT, C], mdt, tag="z2")
                st[b]["z2"] = z2_new
            mvall2 = st[b]["mvall2"]
            rstd2 = st[b]["rstd2"]
            nc.vector.tensor_scalar(out=st[b]["z2"][:, mc, :],
                                    in0=st[b]["fwp"][:, mc, :],
                                    scalar1=mvall2[:, 0:1, mc],
                                    scalar2=rstd2[:, mc:mc + 1],
                                    op0=sub, op1=mult)
            if mc == CT - 1:
                del st[b]["fwp"], st[b]["mvall2"], st[b]["rstd2"]

        def emit_t2_final_group(b, kt):
            z2 = st[b]["z2"]
            xb = st[b]["xb"]
            pt2 = pt2_buf[:, kt % 2, :]
            for mc in range(CT):
                nc.tensor.transpose(pt2[:, mc * P:(mc + 1) * P],
                                    z2[:, mc, kt * P:(kt + 1) * P], id_sb)
            res = resp.tile([P, C], f32, tag="res")
            nc.scalar.activation(out=res, in_=pt2, func=Ident,
                                 bias=gb_sb[:, kt, 1:2],
                                 scale=gb_sb[:, kt, 0:1])
            # final multiply on GpSimd (idle engine; SBUF-only operands) to
            # keep DVE under the PE roofline — except the tail batches,
            # where the 1.4us gpsimd op sits on the drain critical path
            eng = nc.vector if b >= nb - 2 else nc.gpsimd
            eng.tensor_mul(out=res, in0=res, in1=xb[:, kt, :])
            nc.sync.dma_start(out=out_d[b, kt * P:(kt + 1) * P, :], in_=res)
            if kt == KT - 1:
                del st[b]

        # software pipeline, 2-batch skew, with transpose groups woven
        # between independent matmul groups so their psum evictions are
        # hidden behind PE work instead of stalling the pt slots:
        #   cycle b: DCT(b) x T1(b-1) | fc1(b-1) x T2(b-2) | fc2(b-1)
        for b in range(nb + 2):
            if b < nb:
                emit_load(b)
            if b == 0:
                # weights are first needed by fc1/fc2 of cycle 1 — loading
                # them after x(0)/dt keeps the first DCT off the DMA queue's
                # critical path (saves ~10us of head)
                nc.sync.dma_start(out=w1t_sb,
                                  in_=w1t_d.rearrange("(t p) h -> p t h", p=P))
                nc.sync.dma_start(out=w2t_sb,
                                  in_=w2t_d.rearrange("(t p) k -> p t k", p=P))
            # T1 emitted BEFORE the paired DCT group, rotated so the last-
            # needed zT chunk (kt=3) is produced first: fc1's first group no
            # longer waits on the last transpose eviction
            kt_rot = [3, 0, 1, 2]
            for g in range(max(CT, KT)):
                if 1 <= b <= nb:
                    emit_t1_group(b - 1, kt_rot[g])
                if b < nb:
                    emit_dct_group(b, g)
            if b < nb:
                emit_ln1_rstd(b)
                for g in range(CT):
                    emit_ln1_evict(b, g)
            for mh in range(HT):
                if 1 <= b <= nb:
                    emit_fc1_group(b - 1, mh)
                if b >= 2 and mh % 2 == 1:
                    emit_t2_final_group(b - 2, mh // 2)
            if 1 <= b <= nb:
                for g in range(CT):
                    emit_fc2_group(b - 1, g)
                emit_ln2_rstd(b - 1)
                for g in range(CT):
                    emit_ln2_evict(b - 1, g)

    # Bacc's compile passes (register alloc, wait splitting for fp32 matmuls)
    # run in finalize(); the pjrt exec path requires a finalized module.
    nc.finalize()
    return nc


def get_nc(nb: int):
    key = nb
    if key not in _NC_CACHE:
        _NC_CACHE[key] = _build(nb)
    return _NC_CACHE[key]


def make_host_inputs(x, gamma, beta, w1, w2):
    """Host-side precompute: folded-DCT inputs + matrices + weights, bf16."""
    import ml_dtypes
    bf16 = ml_dtypes.bfloat16

    xf32 = np.asarray(x, dtype=np.float32)
    x = np.ascontiguousarray(xf32.astype(bf16))
    gamma = np.asarray(gamma, dtype=np.float32)
    beta = np.asarray(beta, dtype=np.float32)
    w1 = np.asarray(w1, dtype=np.float32)
    w2 = np.asarray(w2, dtype=np.float32)

    # DCT-II_512 = host butterflies + [DCT-II_128(e2) | DCT-IV_128(o2) |
    # DCT-IV_256(o)], outputs k-permuted (absorbed into w1t row order)
    e = xf32[:, :C // 2, :] + xf32[:, :C // 2 - 1:-1, :]
    o = xf32[:, :C // 2, :] - xf32[:, :C // 2 - 1:-1, :]
    e2 = e[:, :C // 4, :] + e[:, :C // 4 - 1:-1, :]
    o2 = e[:, :C // 4, :] - e[:, :C // 4 - 1:-1, :]
    xf = np.ascontiguousarray(
        np.concatenate([e2, o2, o], axis=1).astype(bf16))       # [B, C, C]

    kk = np.arange(P)[:, None].astype(np.float64)
    ll = np.arange(P)[None, :].astype(np.float64)
    M2_128 = 2.0 * np.cos(np.pi * kk * (2 * ll + 1) / (2 * P))
    M4_128 = 2.0 * np.cos(np.pi * (2 * kk + 1) * (2 * ll + 1) / (4 * P))
    kk2 = np.arange(2 * P)[:, None].astype(np.float64)
    ll2 = np.arange(2 * P)[None, :].astype(np.float64)
    M4_256 = 2.0 * np.cos(np.pi * (2 * kk2 + 1) * (2 * ll2 + 1) / (8 * P))
    dm = np.zeros((2 * P, C), dtype=np.float32)
    dm[0:P, 0:P] = M2_128.T
    dm[0:P, P:2 * P] = M4_128.T
    dm[0:P, 2 * P:] = M4_256.T[0:P, :]
    dm[P:2 * P, 2 * P:] = M4_256.T[P:2 * P, :]
    dm = np.ascontiguousarray(dm.astype(bf16))

    # pf column j holds freq[perm[j]] — permute w1g rows to match
    perm = np.concatenate([4 * np.arange(P), 4 * np.arange(P) + 2,
                           2 * np.arange(2 * P) + 1])
    w1t = np.ascontiguousarray(
        (w1 * gamma[None, :]).T[perm, :].astype(bf16))          # [k', h]
    b1 = (w1 @ beta).astype(np.float32)                         # [h]
    w2t = np.ascontiguousarray(w2.T.astype(bf16))               # [h, k]
    gb = np.ascontiguousarray(np.stack([gamma, beta], axis=1))  # [k, 2]
    iden = np.eye(P, dtype=np.float32).astype(bf16)
    return x, xf, dict(dm=dm, w1t=w1t, b1=b1, w2t=w2t, gb=gb, iden=iden)


def make_in_maps(x, xf, const):
    nb = B_FULL // N_CORES
    return [dict(x=x[i * nb:(i + 1) * nb], xf=xf[i * nb:(i + 1) * nb], **const)
            for i in range(N_CORES)]


def kernel(x, gamma, beta, w1, w2):
    import time
    from concourse.bass_utils import run_bass_kernel_spmd

    x, xf, const = make_host_inputs(x, gamma, beta, w1, w2)
    nc = get_nc(B_FULL // N_CORES)
    in_maps = make_in_maps(x, xf, const)
    last_err = None
    for attempt in range(3):
        try:
            r = run_bass_kernel_spmd(nc, in_maps, list(range(N_CORES)))
            return np.concatenate(
                [r.results[i]["out"] for i in range(N_CORES)], axis=0)
        except Exception as e:  # transient device wedge recovers on retry
            last_err = e
            time.sleep(5)
    raise last_err


# revision 39
# speedup vs baseline: 1.0575x; 1.0575x over previous
"""FECAM layer Trainium2 kernel (bf16 matmul version).

Reference computation (per batch element b, X = x[b] in R^{512x512}, layout [l, c]):
    xp   = X^T                                  # [c, l]
    freq = xp @ D^T                             # DCT-II along l      [c, k]
    sd   = LN(freq) * gamma + beta              # LayerNorm over k
    h    = relu(sd @ W1^T)                      # [c, 2C]
    fw   = sigmoid(h @ W2^T)                    # [c, k]
    fw   = LN(fw) * gamma + beta
    out  = (xp * fw)^T = X .* fw^T              # [l, c]  (natural layout)

Device strategy (data parallel, 16 batch elements per core x 8 cores):
  - ALL matmul/transpose operands bf16 (measured end-to-end rel err ~5e-3
    vs 2e-2 gate).  MATMUL stays 1 cyc/row (same as f32r) but LDWEIGHTS
    drops 4x (f32 weights load at 4 cyc/row) so the PE queue never stalls
    on weight loads, and PE transposes drop 1.5 -> 1.0 cyc/row.
  - freq[c,k] = matmul(lhsT=x_b tiles [l,c], rhs=D^T tiles [l,k]) -> psum
  - LN1 stats via bn_stats/bn_aggr per group, aggregated into mvall
    [P, 2, KT]; ONE batched Ln + ONE batched Exp on [P, KT] per LN per
    batch (instead of 2 small ACT ops per group) -> 12 fewer ACT
    instructions per batch.  rstd = Exp(-0.5*Ln(var+eps)).
  - LN1 evict z = (pf - mu)*rstd on DVE tensor_scalar (psum fp32 1x),
    output bf16.
  - LN1 gamma/beta folded into fc1 on host: W1g[h,k]=w1[h,k]*gamma[k],
    b1[h]=sum_k beta[k]*w1[h,k]
  - z transposed 128x128 via PE (bf16, 1 cyc/row) into bf16 psum; evicted
    by ACT copy to zT [k,c]
  - fc1: hT = relu(W1g @ zT + b1) in [h,c] (ACT evict w/ per-part bias)
  - fc2: y = hT^T @ W2^T -> [c,k]; sigmoid = recip_approx_fast(1+Exp(-y))
  - LN2 stats likewise batched; z2 evict on DVE tensor_scalar (SBUF fp32
    2x mode), output bf16; transpose via PE; final affine (gamma/beta
    per-partition) on ACT; multiply by x on DVE
  - emission is software-pipelined with a 2-batch skew so the PE queue
    always has independent matmul work:
      cycle b emits: DCT+LN1(b) | T1(b-1) | fc1(b-1) x T2+final(b-2) | fc2(b-1)
"""

import sys

if "/opt/trn_rl_repo" not in sys.path:
    sys.path.insert(0, "/opt/trn_rl_repo")

import numpy as np

P = 128
C = 512          # channels == seq len == dct size
H = 1024         # hidden
CT = C // P      # 4 c-tiles
KT = C // P      # 4 k-tiles
HT = H // P      # 8 h-tiles
EPS = 1e-6
N_CORES = 8
B_FULL = 128

_NC_CACHE: dict = {}


def _build(nb: int):
    import concourse.bass as bass
    from concourse import bacc
    import concourse.mybir as mybir
    from concourse.tile import TileContext

    f32 = mybir.dt.float32
    bf16 = mybir.dt.bfloat16
    Relu = mybir.ActivationFunctionType.Relu
    Ln = mybir.ActivationFunctionType.Ln
    Exp = mybir.ActivationFunctionType.Exp
    Ident = mybir.ActivationFunctionType.Identity
    mult = mybir.AluOpType.mult
    sub = mybir.AluOpType.subtract

    mdt = bf16

    nc = bacc.Bacc()
    x_d = nc.declare_dram_parameter("x", [nb, C, C], mdt, isOutput=False)
    # xf rows: [e2 (128) | o2 (128) | o (256)] — host-folded DCT butterflies
    xf_d = nc.declare_dram_parameter("xf", [nb, C, C], mdt, isOutput=False)
    # dm row-block 0: [De2T | Do2T | DoT rows 0:128]; block 1: [pad | DoT 128:256]
    dm_d = nc.declare_dram_parameter("dm", [2 * P, C], mdt, isOutput=False)
    w1t_d = nc.declare_dram_parameter("w1t", [C, H], mdt, isOutput=False)
    b1_d = nc.declare_dram_parameter("b1", [H], f32, isOutput=False)
    w2t_d = nc.declare_dram_parameter("w2t", [H, C], mdt, isOutput=False)
    gb_d = nc.declare_dram_parameter("gb", [C, 2], f32, isOutput=False)
    id_d = nc.declare_dram_parameter("iden", [P, P], mdt, isOutput=False)
    out_d = nc.declare_dram_parameter("out", [nb, C, C], f32, isOutput=True)

    with TileContext(nc) as tc, \
            tc.tile_pool(name="consts", bufs=1) as consts, \
            tc.tile_pool(name="xin", bufs=4) as xin, \
            tc.tile_pool(name="work", bufs=2) as work, \
            tc.tile_pool(name="small", bufs=8) as small, \
            tc.tile_pool(name="res", bufs=4) as resp, \
            tc.tile_pool(name="ps_mm", bufs=4, space="PSUM") as ps_mm, \
            tc.tile_pool(name="ps_t", bufs=2, space="PSUM") as ps_t, \
            tc.tile_pool(name="ps_hw", bufs=2, space="PSUM") as ps_hw:

        # one ACT table set covering Ln/Exp/Identity/Copy/Relu: pre-seed it so
        # bacc's availability pass never inserts another load
        from concourse.hw_specs import get_activation_tables
        set_names = list(get_activation_tables(nc.m.arch))
        nc.scalar.add_instruction(mybir.InstLoadActFuncSet(
            name=nc.get_next_instruction_name(),
            act_func_set_id=set_names.index("natural_log_exp_and_others"),
            ins=[], outs=[]))

        dm_sb = consts.tile([P, 2, C], mdt)
        w1t_sb = consts.tile([P, KT, H], mdt)
        w2t_sb = consts.tile([P, HT, C], mdt)
        b1_sb = consts.tile([P, HT], f32)
        nc.sync.dma_start(out=b1_sb, in_=b1_d.rearrange("(t p) -> p t", p=P))
        gb_sb = consts.tile([P, KT, 2], f32)
        nc.sync.dma_start(out=gb_sb, in_=gb_d.rearrange("(t p) g -> p t g", p=P))
        id_sb = consts.tile([P, P], mdt)
        nc.sync.dma_start(out=id_sb, in_=id_d[:])
        eps_sb = consts.tile([P, 1], f32)
        nc.vector.memset(eps_sb, EPS)

        st: dict = {}   # per-batch live tiles

        def emit_load(b):
            xb = xin.tile([P, KT, C], mdt, tag="xb")
            xfb = xin.tile([P, KT, C], mdt, tag="xfb")
            if b == 0:
                # interleave dm/xf chunk loads across DMA queues so the first
                # DCT matmul (needs xfb[:,0] + dm cols 0:128) starts asap;
                # x(0) is only needed by the final multiply two cycles later
                nc.sync.dma_start(out=xfb[:, 0, :], in_=xf_d[b, 0:P, :])
                nc.sync.dma_start(out=dm_sb[:, 0, 0:P], in_=dm_d[0:P, 0:P])
                nc.sync.dma_start(out=dm_sb[:, 0, P:2 * P],
                                  in_=dm_d[0:P, P:2 * P])
                nc.sync.dma_start(out=dm_sb[:, 0, 2 * P:],
                                  in_=dm_d[0:P, 2 * P:])
                nc.sync.dma_start(out=dm_sb[:, 1, :], in_=dm_d[P:2 * P, :])
                for lt in range(1, KT):
                    nc.sync.dma_start(out=xfb[:, lt, :],
                                      in_=xf_d[b, lt * P:(lt + 1) * P, :])
                nc.sync.dma_start(out=xb,
                                  in_=x_d[b].rearrange("(t p) c -> p t c", p=P))
            else:
                nc.sync.dma_start(out=xfb,
                                  in_=xf_d[b].rearrange("(t p) c -> p t c", p=P))
                nc.sync.dma_start(out=xb,
                                  in_=x_d[b].rearrange("(t p) c -> p t c", p=P))
            st[b] = {"xb": xb, "xfb": xfb}

        def emit_dct_group(b, mc):
            """DCT matmul group mc -> psum pf; bn_stats/aggr into mvall."""
            if mc == 0:
                st[b]["pf"] = []
                mvall = small.tile([P, 2, KT], f32, tag="mvall")
                st[b]["mvall"] = mvall
            xfb = st[b]["xfb"]
            pf = ps_mm.tile([P, C], f32, tag="pf")
            st[b]["pf"].append(pf)
            cs = slice(mc * P, (mc + 1) * P)
            # folded DCT: freq[4k''] from e2, freq[4k''+2] from o2,
            # freq[2k'+1] from o (k-permutation absorbed into w1t rows)
            nc.tensor.matmul(pf[:, 0:P], lhsT=xfb[:, 0, cs],
                             rhs=dm_sb[:, 0, 0:P], start=True, stop=True)
            nc.tensor.matmul(pf[:, P:2 * P], lhsT=xfb[:, 1, cs],
                             rhs=dm_sb[:, 0, P:2 * P], start=True, stop=True)
            nc.tensor.matmul(pf[:, 2 * P:], lhsT=xfb[:, 2, cs],
                             rhs=dm_sb[:, 0, 2 * P:], start=True, stop=False)
            nc.tensor.matmul(pf[:, 2 * P:], lhsT=xfb[:, 3, cs],
                             rhs=dm_sb[:, 1, 2 * P:], start=False, stop=True)
            stats = small.tile([P, 6], f32, tag="stats")
            nc.vector.bn_stats(out=stats, in_=pf)
            nc.vector.bn_aggr(out=st[b]["mvall"][:, :, mc], in_=stats)

        def emit_ln1_rstd(b):
            """Batched rstd for all 4 groups: [P,KT] Ln + Exp."""
            mvall = st[b]["mvall"]
            lv = small.tile([P, KT], f32, tag="lv")
            nc.scalar.activation(out=lv, in_=mvall[:, 1, :], func=Ln,
                                 bias=eps_sb, scale=1.0)
            rstd = small.tile([P, KT], f32, tag="rstd")
            nc.scalar.activation(out=rstd, in_=lv, func=Exp,
                                 bias=0.0, scale=-0.5)
            st[b]["rstd"] = rstd

        def emit_ln1_evict(b, mc):
            if mc == 0:
                z_new = work.tile([P, CT, C], mdt, tag="z")
                st[b]["z"] = z_new
            mvall = st[b]["mvall"]
            rstd = st[b]["rstd"]
            nc.vector.tensor_scalar(out=st[b]["z"][:, mc, :],
                                    in0=st[b]["pf"][mc],
                                    scalar1=mvall[:, 0:1, mc], scalar2=rstd[:, mc:mc + 1],
                                    op0=sub, op1=mult)
            if mc == CT - 1:
                del st[b]["pf"], st[b]["mvall"], st[b]["rstd"]

        def emit_t1_group(b, kt):
            if "zT" not in st[b]:
                zT_new = work.tile([P, KT, C], mdt, tag="zT")
                st[b]["zT"] = zT_new
                st[b]["t1done"] = 0
            z = st[b]["z"]
            zT = st[b]["zT"]
            pt = ps_t.tile([P, C], mdt, tag="pt")
            for mc in range(CT):
                nc.tensor.transpose(pt[:, mc * P:(mc + 1) * P],
                                    z[:, mc, kt * P:(kt + 1) * P], id_sb)
            nc.scalar.copy(out=zT[:, kt, :], in_=pt)
            st[b]["t1done"] += 1
            if st[b]["t1done"] == KT:
                del st[b]["z"]
                del st[b]["t1done"]

        def emit_fc1_group(b, mh):
            if mh == 0:
                hT_new = work.tile([P, HT, C], mdt, tag="hT")
                st[b]["hT"] = hT_new
            zT = st[b]["zT"]
            hT = st[b]["hT"]
            ph = ps_hw.tile([P, C], f32, tag="phw")
            for kt in range(KT):
                nc.tensor.matmul(
                    ph,
                    lhsT=w1t_sb[:, kt, mh * P:(mh + 1) * P],
                    rhs=zT[:, kt, :],
                    start=(kt == 0),
                    stop=(kt == KT - 1),
                )
            nc.scalar.activation(out=hT[:, mh, :], in_=ph, func=Relu,
                                 bias=b1_sb[:, mh:mh + 1], scale=1.0)
            if mh == HT - 1:
                del st[b]["zT"]

        def emit_fc2_group(b, mc):
            """fc2 matmuls + sigmoid + bn stats for group mc."""
            if mc == 0:
                fwp_new = work.tile([P, CT, C], f32, tag="fwp")
                st[b]["fwp"] = fwp_new
                mvall2 = small.tile([P, 2, KT], f32, tag="mvall")
                st[b]["mvall2"] = mvall2
            hT = st[b]["hT"]
            pw = ps_hw.tile([P, C], f32, tag="phw")
            for ht in range(HT):
                nc.tensor.matmul(
                    pw,
                    lhsT=hT[:, ht, mc * P:(mc + 1) * P],
                    rhs=w2t_sb[:, ht, :],
                    start=(ht == 0),
                    stop=(ht == HT - 1),
                )
            fwp = st[b]["fwp"]
            nc.scalar.activation(out=fwp[:, mc, :], in_=pw, func=Exp,
                                 bias=0.0, scale=-1.0)
            nc.vector.tensor_scalar_add(out=fwp[:, mc, :], in0=fwp[:, mc, :],
                                        scalar1=1.0)
            nc.vector.reciprocal_approx_fast(out=fwp[:, mc, :], in_=fwp[:, mc, :])
            stats2 = small.tile([P, 6], f32, tag="stats")
            nc.vector.bn_stats(out=stats2, in_=fwp[:, mc, :])
            nc.vector.bn_aggr(out=st[b]["mvall2"][:, :, mc], in_=stats2)
            if mc == CT - 1:
                del st[b]["hT"]

        def emit_ln2_rstd(b):
            mvall2 = st[b]["mvall2"]
            lv = small.tile([P, KT], f32, tag="lv")
            nc.scalar.activation(out=lv, in_=mvall2[:, 1, :], func=Ln,
                                 bias=eps_sb, scale=1.0)
            rstd2 = small.tile([P, KT], f32, tag="rstd")
            nc.scalar.activation(out=rstd2, in_=lv, func=Exp,
                                 bias=0.0, scale=-0.5)
            st[b]["rstd2"] = rstd2

        def emit_ln2_evict(b, mc):
            if mc == 0:
                z2_new = work.tile([P, CT, C], mdt, tag="z2")
                st[b]["z2"] = z2_new
            mvall2 = st[b]["mvall2"]
            rstd2 = st[b]["rstd2"]
            nc.vector.tensor_scalar(out=st[b]["z2"][:, mc, :],
                                    in0=st[b]["fwp"][:, mc, :],
                                    scalar1=mvall2[:, 0:1, mc],
                                    scalar2=rstd2[:, mc:mc + 1],
                                    op0=sub, op1=mult)
            if mc == CT - 1:
                del st[b]["fwp"], st[b]["mvall2"], st[b]["rstd2"]

        def emit_t2_final_group(b, kt):
            z2 = st[b]["z2"]
            xb = st[b]["xb"]
            pt2 = ps_t.tile([P, C], mdt, tag="pt")
            for mc in range(CT):
                nc.tensor.transpose(pt2[:, mc * P:(mc + 1) * P],
                                    z2[:, mc, kt * P:(kt + 1) * P], id_sb)
            res = resp.tile([P, C], f32, tag="res")
            nc.scalar.activation(out=res, in_=pt2, func=Ident,
                                 bias=gb_sb[:, kt, 1:2],
                                 scale=gb_sb[:, kt, 0:1])
            # final multiply on GpSimd (idle engine; SBUF-only operands) to
            # keep DVE under the PE roofline — except the tail batches,
            # where the 1.4us gpsimd op sits on the drain critical path
            eng = nc.vector if b >= nb - 2 else nc.gpsimd
            eng.tensor_mul(out=res, in0=res, in1=xb[:, kt, :])
            nc.sync.dma_start(out=out_d[b, kt * P:(kt + 1) * P, :], in_=res)
            if kt == KT - 1:
                del st[b]

        # software pipeline, 2-batch skew, with transpose groups woven
        # between independent matmul groups so their psum evictions are
        # hidden behind PE work instead of stalling the pt slots:
        #   cycle b: DCT(b) x T1(b-1) | fc1(b-1) x T2(b-2) | fc2(b-1)
        for b in range(nb + 2):
            if b < nb:
                emit_load(b)
            if b == 0:
                # weights are first needed by fc1/fc2 of cycle 1 — loading
                # them after x(0)/dt keeps the first DCT off the DMA queue's
                # critical path (saves ~10us of head)
                nc.sync.dma_start(out=w1t_sb,
                                  in_=w1t_d.rearrange("(t p) h -> p t h", p=P))
                nc.sync.dma_start(out=w2t_sb,
                                  in_=w2t_d.rearrange("(t p) k -> p t k", p=P))
            # T1 emitted BEFORE the paired DCT group, rotated so the last-
            # needed zT chunk (kt=3) is produced first: fc1's first group no
            # longer waits on the last transpose eviction
            kt_rot = [3, 0, 1, 2]
            for g in range(max(CT, KT)):
                if 1 <= b <= nb:
                    emit_t1_group(b - 1, kt_rot[g])
                if b < nb:
                    emit_dct_group(b, g)
            if b < nb:
                emit_ln1_rstd(b)
                for g in range(CT):
                    emit_ln1_evict(b, g)
            for mh in range(HT):
                if 1 <= b <= nb:
                    emit_fc1_group(b - 1, mh)
                if b >= 2 and mh % 2 == 1:
                    emit_t2_final_group(b - 2, mh // 2)
            if 1 <= b <= nb:
                for g in range(CT):
                    emit_fc2_group(b - 1, g)
                emit_ln2_rstd(b - 1)
                for g in range(CT):
                    emit_ln2_evict(b - 1, g)

    # Bacc's compile passes (register alloc, wait splitting for fp32 matmuls)
    # run in finalize(); the pjrt exec path requires a finalized module.
    nc.finalize()
    return nc


def get_nc(nb: int):
    key = nb
    if key not in _NC_CACHE:
        _NC_CACHE[key] = _build(nb)
    return _NC_CACHE[key]


def make_host_inputs(x, gamma, beta, w1, w2):
    """Host-side precompute: folded-DCT inputs + matrices + weights, bf16."""
    import ml_dtypes
    bf16 = ml_dtypes.bfloat16

    xf32 = np.asarray(x, dtype=np.float32)
    x = np.ascontiguousarray(xf32.astype(bf16))
    gamma = np.asarray(gamma, dtype=np.float32)
    beta = np.asarray(beta, dtype=np.float32)
    w1 = np.asarray(w1, dtype=np.float32)
    w2 = np.asarray(w2, dtype=np.float32)

    # DCT-II_512 = host butterflies + [DCT-II_128(e2) | DCT-IV_128(o2) |
    # DCT-IV_256(o)], outputs k-permuted (absorbed into w1t row order)
    e = xf32[:, :C // 2, :] + xf32[:, :C // 2 - 1:-1, :]
    o = xf32[:, :C // 2, :] - xf32[:, :C // 2 - 1:-1, :]
    e2 = e[:, :C // 4, :] + e[:, :C // 4 - 1:-1, :]
    o2 = e[:, :C // 4, :] - e[:, :C // 4 - 1:-1, :]
    xf = np.ascontiguousarray(
        np.concatenate([e2, o2, o], axis=1).astype(bf16))       # [B, C, C]

    kk = np.arange(P)[:, None].astype(np.float64)
    ll = np.arange(P)[None, :].astype(np.float64)
    M2_128 = 2.0 * np.cos(np.pi * kk * (2 * ll + 1) / (2 * P))
    M4_128 = 2.0 * np.cos(np.pi * (2 * kk + 1) * (2 * ll + 1) / (4 * P))
    kk2 = np.arange(2 * P)[:, None].astype(np.float64)
    ll2 = np.arange(2 * P)[None, :].astype(np.float64)
    M4_256 = 2.0 * np.cos(np.pi * (2 * kk2 + 1) * (2 * ll2 + 1) / (8 * P))
    dm = np.zeros((2 * P, C), dtype=np.float32)
    dm[0:P, 0:P] = M2_128.T
    dm[0:P, P:2 * P] = M4_128.T
    dm[0:P, 2 * P:] = M4_256.T[0:P, :]
    dm[P:2 * P, 2 * P:] = M4_256.T[P:2 * P, :]
    dm = np.ascontiguousarray(dm.astype(bf16))

    # pf column j holds freq[perm[j]] — permute w1g rows to match
    perm = np.concatenate([4 * np.arange(P), 4 * np.arange(P) + 2,
                           2 * np.arange(2 * P) + 1])
    w1t = np.ascontiguousarray(
        (w1 * gamma[None, :]).T[perm, :].astype(bf16))          # [k', h]
    b1 = (w1 @ beta).astype(np.float32)                         # [h]
    w2t = np.ascontiguousarray(w2.T.astype(bf16))               # [h, k]
    gb = np.ascontiguousarray(np.stack([gamma, beta], axis=1))  # [k, 2]
    iden = np.eye(P, dtype=np.float32).astype(bf16)
    return x, xf, dict(dm=dm, w1t=w1t, b1=b1, w2t=w2t, gb=gb, iden=iden)


def make_in_maps(x, xf, const):
    nb = B_FULL // N_CORES
    return [dict(x=x[i * nb:(i + 1) * nb], xf=xf[i * nb:(i + 1) * nb], **const)
            for i in range(N_CORES)]


def kernel(x, gamma, beta, w1, w2):
    import time
    from concourse.bass_utils import run_bass_kernel_spmd

    x, xf, const = make_host_inputs(x, gamma, beta, w1, w2)
    nc = get_nc(B_FULL // N_CORES)
    in_maps = make_in_maps(x, xf, const)
    last_err = None
    for attempt in range(3):
        try:
            r = run_bass_kernel_spmd(nc, in_maps, list(range(N_CORES)))
            return np.concatenate(
                [r.results[i]["out"] for i in range(N_CORES)], axis=0)
        except Exception as e:  # transient device wedge recovers on retry
            last_err = e
            time.sleep(5)
    raise last_err
